# revision 26
# baseline (speedup 1.0000x reference)
"""Multi-head attention kernel for Trainium2 (Bass/Tile), 8-core SPMD.

Problem: Q,K,V [B=2, H=16, S=4096, D=64] fp32 -> softmax(Q K^T / sqrt(D)) V.
Sharding: batch*heads (32) split 4-per-core across 8 NeuronCores; each core
computes its heads independently (no collectives).

Per-head algorithm (transposed-scores flash attention, fp16 matmuls, P
production split across two engines):
  scoresT[k,q] = K[k,:] . Q[q,:]        (PE, fp16 operands, fp32 PSUM,
                                         row-tiled pairs: two k-chunks run
                                         concurrently in the 128x128 array)
  pT[k,q]     = exp(scoresT / 8)        (fp16; per 3-chunk group either ACT
                                         exact exp, or DVE fp16-Schraudolph:
                                         u16 = round(s*1024*log2(e)/8 + b)
                                         bitcast as fp16 == 2^x with a
                                         mean-compensated ~+-3% mantissa-
                                         interpolation sawtooth that washes
                                         out in the softmax normalization)
  accT[d,q]  += Vaug[k,d] . pT[k,q]     (PE fp16; Vaug row 64 == ones, so acc
                                         row 64 accumulates the denominator)
  The raw accumulator [65, qtile] (numerator rows 0..63 + denominator row 64)
  is pulled PSUM->SBUF by one tracked DVE copy and DMA'd to DRAM in fp32;
  the softmax division happens on the HOST during the gather (removes the
  reciprocal/broadcast/multiply epilogue from the device critical path).
Host side re-lays-out data: QT/KT transposed per head, V augmented with
a ones column; output acc gathered, divided, transposed back.
"""

import numpy as np
from contextlib import ExitStack

import concourse.bacc as bacc
import concourse.bass as bass
import concourse.tile as tile
import concourse.mybir as mybir
from concourse.bass_utils import run_bass_kernel_spmd

F32 = mybir.dt.float32
F16 = mybir.dt.float16
U16 = mybir.dt.uint16
EXP = mybir.ActivationFunctionType.Exp
MULT = mybir.AluOpType.mult
ADD = mybir.AluOpType.add

B, H, S, D = 2, 16, 4096, 64
N_CORES = 8
HPC = (B * H) // N_CORES  # heads per core

QTILE = 512            # q columns processed per inner iteration
CHUNK = 128            # k rows per matmul (PE partition dim)
GROUP = 2              # k-chunks exp'd per ACT/DVE instruction

# fp16 Schraudolph: u16 = round(score * 1024*log2(e)/8 + bias); the bias is
# 15*1024 minus 58.7 to zero the mean of the 2^f vs (1+f) mantissa sawtooth
DVE_MUL = 184.66496
DVE_ADD = 15301.3
# groups (of 16 per qtile) whose P runs on DVE instead of ACT (16/32 chunks);
# ALL odd groups -> every 2-group block pairs exactly one ACT with one DVE
# producer (two consecutive same-engine groups serialize ~2.2us of exp in a
# ~1.5us block and stall the next qtile's MM1 on the sc-buffer recycle)
DVE_GROUPS = frozenset({1, 3, 5, 7, 9, 11, 13, 15})
FLUSH_DEPTH = 3        # MM2 groups held back so PE never queues behind P


def build_nc(hpc: int = HPC, s: int = S, qtile: int = QTILE):
    n_chunks = s // CHUNK
    n_qtiles = s // qtile
    group_sizes = [GROUP] * (n_chunks // GROUP)
    if n_chunks % GROUP:
        group_sizes.append(n_chunks % GROUP)

    nc = bacc.Bacc("TRN2", target_bir_lowering=False, debug=False)
    qt_d = nc.dram_tensor("qt", [hpc, D, s], F16, kind="ExternalInput").ap()
    kt_d = nc.dram_tensor("kt", [hpc, D, s], F16, kind="ExternalInput").ap()
    va_d = nc.dram_tensor("va", [hpc, s, D + 1], F16, kind="ExternalInput").ap()
    o_d = nc.dram_tensor(
        "o", [hpc, s // qtile, D + 1, qtile], F32, kind="ExternalOutput"
    ).ap()

    with tile.TileContext(nc) as tc, ExitStack() as ctx:
        qk_pool = ctx.enter_context(tc.tile_pool(name="qk", bufs=2))
        v_pool = ctx.enter_context(tc.tile_pool(name="v", bufs=2))
        pt_pool = ctx.enter_context(tc.tile_pool(name="pt", bufs=6))
        oc_pool = ctx.enter_context(tc.tile_pool(name="oc", bufs=3))
        const_pool = ctx.enter_context(tc.tile_pool(name="const", bufs=1))
        sc_psum = ctx.enter_context(tc.tile_pool(name="sc", bufs=3, space="PSUM"))
        oa_psum = ctx.enter_context(tc.tile_pool(name="oa", bufs=2, space="PSUM"))

        # prewarm the PE HAM clock gate: the hardware activity monitor only
        # releases the 1.2->2.4 GHz throttle after a full ~3.4us window of
        # sustained PE activity, which otherwise burns the first ~dozens of
        # real matmuls at half clock; ~4.5us of junk micro-matmuls into a
        # scratch PSUM tile keep the PE busy through the initial DMA window
        # so real MM1s start warm
        wj = const_pool.tile([64, 128], F16)
        nc.vector.memset(wj[:], 1.0)
        # prewarm the ACT exp table set while the first DMAs are in flight
        warm = const_pool.tile([1, 1], F32)
        nc.vector.memset(warm[:], 0.0)
        warm2 = const_pool.tile([1, 1], F32)
        nc.scalar.activation(warm2[:], warm[:], EXP, scale=1.0)
        # (tag "sc": shares the sc ring's buffers, so no extra PSUM banks)
        warm_ps = sc_psum.tile([16, 128], F32, tag="sc")
        for _ in range(42):
            nc.tensor.matmul(
                warm_ps[:], wj[:, 0:16], wj[:], start=True, stop=True,
                skip_group_check=True,
            )

        # software-pipelined emission: each group's PV matmuls (MM2) are
        # deferred until after the NEXT group's score matmuls (MM1) and P
        # production -- across qtile AND head boundaries -- so the next P is
        # never queued on PE behind MM2s that wait on the current P.
        pending = []  # deque of (h, qt, acc, chunks, p_t, va_sb, is_last)

        def epilogue(h_, qt_, acc_):
            # pull the raw accumulator (numerator + Z row) out of PSUM with a
            # *tracked* DVE copy (waits for the accumulation stop), then ship
            # it fp32 to DRAM on the HWDGE ring; the softmax division happens
            # host-side during the gather.
            oc = oc_pool.tile([D + 1, qtile], F32, tag="oc")
            if h_ == hpc - 1 and qt_ == s // qtile - 1:
                # final qtile is the serial tail of the whole kernel: split
                # the copy/DMA in half so the first DMA overlaps the second
                # copy
                half = qtile // 2
                nc.vector.tensor_copy(oc[:, 0:half], acc_[:, 0:half])
                nc.sync.dma_start(o_d[h_][qt_][:, 0:half], oc[:, 0:half])
                nc.vector.tensor_copy(oc[:, half:], acc_[:, half:])
                nc.sync.dma_start(o_d[h_][qt_][:, half:], oc[:, half:])
            else:
                nc.vector.tensor_copy(oc[:], acc_[:])
                nc.sync.dma_start(o_d[h_][qt_], oc[:])

        def flush_one():
            # NOTE: padding the weight AP to a full 128 columns (output
            # partitions 65..127 as junk) was tried to get fast-weight-load:
            # it makes the whole PE run 1.2x SLOWER -- the extra array
            # columns burn real power and trip the P0 downclock (2.4->2.0
            # GHz). Keep the weight tile at its true 65 columns.
            h_, qt_, acc_, chunks_, pt_, va_, last_ = pending.pop(0)
            for j, c in enumerate(chunks_):
                nc.tensor.matmul(
                    acc_[:],
                    va_[:, c * (D + 1) : (c + 1) * (D + 1)],
                    pt_[:, j, :],
                    start=(c == 0), stop=(c == n_chunks - 1),
                )
            if last_:
                epilogue(h_, qt_, acc_)

        def flush_pending(depth=0):
            while len(pending) > depth:
                flush_one()

        for h in range(hpc):
            # K^T and Q^T [D, s] duplicated into both partition halves so two
            # k-chunks can run concurrently via PE row tiling.
            qt_sb = qk_pool.tile([128, s], F16, tag="qt")
            kt_sb = qk_pool.tile([128, s], F16, tag="kt")
            va_sb = v_pool.tile([128, n_chunks * (D + 1)], F16)
            va_r = va_d[h].rearrange("(c p) e -> p c e", p=128)
            # tiered loads: small leading slices of everything first (just
            # enough for the first block: 4 k-chunks + first qtile of Q),
            # split across both HWDGE queues (sync + scalar) so the critical
            # first-wave descriptors dispatch in parallel; then interleaved
            # k/V column pieces, with the q tails (needed only from qtile 1)
            # last
            kcut = min(4 * CHUNK, s)
            ncut = kcut // CHUNK
            nc.sync.dma_start(kt_sb[0:D, 0:kcut], kt_d[h][:, 0:kcut])
            nc.scalar.dma_start(kt_sb[D : 2 * D, 0:kcut], kt_d[h][:, 0:kcut])
            nc.sync.dma_start(qt_sb[0:D, 0:qtile], qt_d[h][:, 0:qtile])
            nc.scalar.dma_start(qt_sb[D : 2 * D, 0:qtile], qt_d[h][:, 0:qtile])
            nc.sync.dma_start(
                va_sb[:, 0 : ncut * (D + 1)], va_r[:, 0:ncut, :]
            )
            cuts = [kcut]
            while cuts[-1] < s:
                cuts.append(min(cuts[-1] + 12 * CHUNK, s))
            for c0_, c1_ in zip(cuts, cuts[1:]):
                n0_, n1_ = c0_ // CHUNK, c1_ // CHUNK
                nc.sync.dma_start(kt_sb[0:D, c0_:c1_], kt_d[h][:, c0_:c1_])
                nc.sync.dma_start(kt_sb[D : 2 * D, c0_:c1_], kt_d[h][:, c0_:c1_])
                nc.sync.dma_start(
                    va_sb[:, n0_ * (D + 1) : n1_ * (D + 1)], va_r[:, n0_:n1_, :]
                )
            if qtile < s:
                nc.sync.dma_start(qt_sb[0:D, qtile:s], qt_d[h][:, qtile:s])
                nc.sync.dma_start(qt_sb[D : 2 * D, qtile:s], qt_d[h][:, qtile:s])

            for qt in range(n_qtiles):
                qs = slice(qt * qtile, (qt + 1) * qtile)
                acc = oa_psum.tile([D + 1, qtile], F32)
                n_groups = len(group_sizes)
                # emit in blocks of two groups: MM1 x4, then both P ops (one
                # ACT + one DVE, concurrent), then older blocks' MM2s.
                # (3-group blocks were tried to amortize the ~97ns weight-
                # switch bubble over longer runs: slower overall -- with
                # sc bufs=3 a 3-group burst leaves zero pipeline slack and
                # the sc-recycle stalls cost more than the saved switches.)
                block_list = [
                    list(range(i, min(i + 2, n_groups)))
                    for i in range(0, n_groups, 2)
                ]
                for blk_groups in block_list:
                    blk = []
                    for gi in blk_groups:
                        gs = group_sizes[gi]
                        chunks = list(range(GROUP * gi, GROUP * gi + gs))
                        sc = sc_psum.tile([128, GROUP, qtile], F32, tag="sc")
                        for j, c in enumerate(chunks):
                            # alternate partition halves by global chunk index
                            # so consecutive chunks always run concurrently
                            # via PE row tiling, across group boundaries too
                            half = slice(0, D) if c % 2 == 0 else slice(D, 2 * D)
                            nc.tensor.matmul(
                                sc[:, j, :],
                                kt_sb[half, c * CHUNK : (c + 1) * CHUNK],
                                qt_sb[half, qs],
                                start=True, stop=True,
                            )
                        blk.append((gi, gs, chunks, sc))
                    for gi, gs, chunks, sc in blk:
                        p_t = pt_pool.tile([128, GROUP, qtile], F16, tag="pt")
                        if gi in DVE_GROUPS:
                            nc.vector.tensor_scalar(
                                p_t.bitcast(U16)[:, 0:gs, :], sc[:, 0:gs, :],
                                DVE_MUL, DVE_ADD, MULT, ADD,
                            )
                        else:
                            nc.scalar.activation(
                                p_t[:, 0:gs, :], sc[:, 0:gs, :], EXP, scale=0.125
                            )
                        pending.append((
                            h, qt, acc, chunks, p_t, va_sb,
                            gi == n_groups - 1,
                        ))
                    flush_pending(FLUSH_DEPTH)
        flush_pending()

    nc.compile()
    return nc


_NC_CACHE = {}


def _get_nc(hpc=HPC, s=S, qtile=QTILE):
    key = (hpc, s, qtile)
    if key not in _NC_CACHE:
        _NC_CACHE[key] = build_nc(hpc, s, qtile)
    return _NC_CACHE[key]


def prep_inputs(Q, K, V):
    """Host-side re-layout: per-core input maps."""
    bh = B * H
    q2 = np.ascontiguousarray(
        np.asarray(Q, dtype=np.float32).reshape(bh, S, D).transpose(0, 2, 1)
    ).astype(np.float16)
    k2 = np.ascontiguousarray(
        np.asarray(K, dtype=np.float32).reshape(bh, S, D).transpose(0, 2, 1)
    ).astype(np.float16)
    v = np.asarray(V, dtype=np.float32).reshape(bh, S, D).astype(np.float16)
    va = np.concatenate([v, np.ones((bh, S, 1), dtype=np.float16)], axis=-1)
    in_maps = []
    for c in range(N_CORES):
        sl = slice(c * HPC, (c + 1) * HPC)
        in_maps.append({
            "qt": np.ascontiguousarray(q2[sl]),
            "kt": np.ascontiguousarray(k2[sl]),
            "va": np.ascontiguousarray(va[sl]),
        })
    return in_maps


def run(Q, K, V, trace=False, **kwargs):
    nc = _get_nc()
    in_maps = prep_inputs(Q, K, V)
    res = run_bass_kernel_spmd(
        nc, in_maps, core_ids=list(range(N_CORES)), trace=trace, **kwargs
    )
    # o is [hpc, n_qtiles, D+1, qtile] fp32 raw accumulators; divide by the
    # Z row (softmax denominator) and transpose back to [hpc, s, D]
    outs = []
    for c in range(N_CORES):
        o = res.results[c]["o"]  # [hpc, nq, 65, qtile]
        out = o[:, :, 0:D, :] / o[:, :, D : D + 1, :]
        # [hpc, nq, D, qtile] -> [hpc, nq, qtile, D] -> [hpc, s, D]
        outs.append(out.transpose(0, 1, 3, 2).reshape(HPC, S, D))
    full = np.concatenate(outs, axis=0).reshape(B, H, S, D)
    return full, res


def kernel(Q, K, V):
    # retry on transient device/runtime errors (e.g. a wedged NeuronCore
    # left over from a previous run recovers on re-execution)
    import time
    last = None
    for attempt in range(3):
        try:
            out, _ = run(Q, K, V)
            return out
        except Exception as e:  # noqa: BLE001
            last = e
            time.sleep(5)
    raise last



# revision 28
# speedup vs baseline: 1.0048x; 1.0048x over previous
"""Multi-head attention kernel for Trainium2 (Bass/Tile), 8-core SPMD.

Problem: Q,K,V [B=2, H=16, S=4096, D=64] fp32 -> softmax(Q K^T / sqrt(D)) V.
Sharding: batch*heads (32) split 4-per-core across 8 NeuronCores; each core
computes its heads independently (no collectives).

Per-head algorithm (transposed-scores flash attention, fp16 matmuls, P
production split across two engines):
  scoresT[k,q] = K[k,:] . Q[q,:]        (PE, fp16 operands, fp32 PSUM,
                                         row-tiled pairs: two k-chunks run
                                         concurrently in the 128x128 array)
  pT[k,q]     = exp(scoresT / 8)        (fp16; per 3-chunk group either ACT
                                         exact exp, or DVE fp16-Schraudolph:
                                         u16 = round(s*1024*log2(e)/8 + b)
                                         bitcast as fp16 == 2^x with a
                                         mean-compensated ~+-3% mantissa-
                                         interpolation sawtooth that washes
                                         out in the softmax normalization)
  accT[d,q]  += Vaug[k,d] . pT[k,q]     (PE fp16; Vaug row 64 == ones, so acc
                                         row 64 accumulates the denominator)
  The raw accumulator [65, qtile] (numerator rows 0..63 + denominator row 64)
  is pulled PSUM->SBUF by one tracked DVE copy and DMA'd to DRAM in fp32;
  the softmax division happens on the HOST during the gather (removes the
  reciprocal/broadcast/multiply epilogue from the device critical path).
Host side re-lays-out data: QT/KT transposed per head, V augmented with
a ones column; output acc gathered, divided, transposed back.
"""

import numpy as np
from contextlib import ExitStack

import concourse.bacc as bacc
import concourse.bass as bass
import concourse.tile as tile
import concourse.mybir as mybir
from concourse.bass_utils import run_bass_kernel_spmd

F32 = mybir.dt.float32
F16 = mybir.dt.float16
U16 = mybir.dt.uint16
EXP = mybir.ActivationFunctionType.Exp
MULT = mybir.AluOpType.mult
ADD = mybir.AluOpType.add

B, H, S, D = 2, 16, 4096, 64
N_CORES = 8
HPC = (B * H) // N_CORES  # heads per core

QTILE = 512            # q columns processed per inner iteration
CHUNK = 128            # k rows per matmul (PE partition dim)
GROUP = 2              # k-chunks exp'd per ACT/DVE instruction

# fp16 Schraudolph: u16 = round(score * 1024*log2(e)/8 + bias); the bias is
# 15*1024 minus 58.7 to zero the mean of the 2^f vs (1+f) mantissa sawtooth
DVE_MUL = 184.66496
DVE_ADD = 15301.3
# groups (of 16 per qtile) whose P runs on DVE instead of ACT (16/32 chunks);
# ALL odd groups -> every 2-group block pairs exactly one ACT with one DVE
# producer (two consecutive same-engine groups serialize ~2.2us of exp in a
# ~1.5us block and stall the next qtile's MM1 on the sc-buffer recycle)
DVE_GROUPS = frozenset({1, 3, 5, 7, 9, 11, 13, 15})
FLUSH_DEPTH = 2        # MM2 groups held back so PE never queues behind P


def build_nc(hpc: int = HPC, s: int = S, qtile: int = QTILE):
    n_chunks = s // CHUNK
    n_qtiles = s // qtile
    group_sizes = [GROUP] * (n_chunks // GROUP)
    if n_chunks % GROUP:
        group_sizes.append(n_chunks % GROUP)

    nc = bacc.Bacc("TRN2", target_bir_lowering=False, debug=False)
    qt_d = nc.dram_tensor("qt", [hpc, D, s], F16, kind="ExternalInput").ap()
    kt_d = nc.dram_tensor("kt", [hpc, D, s], F16, kind="ExternalInput").ap()
    va_d = nc.dram_tensor("va", [hpc, s, D + 1], F16, kind="ExternalInput").ap()
    o_d = nc.dram_tensor(
        "o", [hpc, s // qtile, D + 1, qtile], F32, kind="ExternalOutput"
    ).ap()

    with tile.TileContext(nc) as tc, ExitStack() as ctx:
        qk_pool = ctx.enter_context(tc.tile_pool(name="qk", bufs=2))
        v_pool = ctx.enter_context(tc.tile_pool(name="v", bufs=2))
        pt_pool = ctx.enter_context(tc.tile_pool(name="pt", bufs=6))
        oc_pool = ctx.enter_context(tc.tile_pool(name="oc", bufs=3))
        const_pool = ctx.enter_context(tc.tile_pool(name="const", bufs=1))
        sc_psum = ctx.enter_context(tc.tile_pool(name="sc", bufs=3, space="PSUM"))
        oa_psum = ctx.enter_context(tc.tile_pool(name="oa", bufs=2, space="PSUM"))

        # prewarm the PE HAM clock gate: the hardware activity monitor only
        # releases the 1.2->2.4 GHz throttle after a full ~3.4us window of
        # sustained PE activity, which otherwise burns the first ~dozens of
        # real matmuls at half clock; ~4.5us of junk micro-matmuls into a
        # scratch PSUM tile keep the PE busy through the initial DMA window
        # so real MM1s start warm
        wj = const_pool.tile([128, 128], F16)
        nc.vector.memset(wj[:], 1.0)
        # prewarm the ACT exp table set while the first DMAs are in flight
        warm = const_pool.tile([1, 1], F32)
        nc.vector.memset(warm[:], 0.0)
        warm2 = const_pool.tile([1, 1], F32)
        nc.scalar.activation(warm2[:], warm[:], EXP, scale=1.0)
        # full-array (128x128) junk matmuls: the HAM watches array
        # utilization, so narrow warmup MMs don't register as busy
        # (tag "sc": shares the sc ring's buffers, so no extra PSUM banks)
        warm_ps = sc_psum.tile([128, 128], F32, tag="sc")
        for _ in range(42):
            nc.tensor.matmul(
                warm_ps[:], wj[:], wj[:], start=True, stop=True,
                skip_group_check=True,
            )

        # software-pipelined emission: each group's PV matmuls (MM2) are
        # deferred until after the NEXT group's score matmuls (MM1) and P
        # production -- across qtile AND head boundaries -- so the next P is
        # never queued on PE behind MM2s that wait on the current P.
        pending = []  # deque of (h, qt, acc, chunks, p_t, va_sb, is_last)

        def epilogue(h_, qt_, acc_):
            # pull the raw accumulator (numerator + Z row) out of PSUM with a
            # *tracked* DVE copy (waits for the accumulation stop), then ship
            # it fp32 to DRAM on the HWDGE ring; the softmax division happens
            # host-side during the gather.
            oc = oc_pool.tile([D + 1, qtile], F32, tag="oc")
            if h_ == hpc - 1 and qt_ == s // qtile - 1:
                # final qtile is the serial tail of the whole kernel: split
                # the copy/DMA in half so the first DMA overlaps the second
                # copy
                half = qtile // 2
                nc.vector.tensor_copy(oc[:, 0:half], acc_[:, 0:half])
                nc.sync.dma_start(o_d[h_][qt_][:, 0:half], oc[:, 0:half])
                nc.vector.tensor_copy(oc[:, half:], acc_[:, half:])
                nc.sync.dma_start(o_d[h_][qt_][:, half:], oc[:, half:])
            else:
                nc.vector.tensor_copy(oc[:], acc_[:])
                nc.sync.dma_start(o_d[h_][qt_], oc[:])

        def flush_one():
            # NOTE: padding the weight AP to a full 128 columns (output
            # partitions 65..127 as junk) was tried to get fast-weight-load:
            # it makes the whole PE run 1.2x SLOWER -- the extra array
            # columns burn real power and trip the P0 downclock (2.4->2.0
            # GHz). Keep the weight tile at its true 65 columns.
            h_, qt_, acc_, chunks_, pt_, va_, last_ = pending.pop(0)
            for j, c in enumerate(chunks_):
                nc.tensor.matmul(
                    acc_[:],
                    va_[:, c * (D + 1) : (c + 1) * (D + 1)],
                    pt_[:, j, :],
                    start=(c == 0), stop=(c == n_chunks - 1),
                )
            if last_:
                epilogue(h_, qt_, acc_)

        def flush_pending(depth=0):
            while len(pending) > depth:
                flush_one()

        for h in range(hpc):
            # K^T and Q^T [D, s] duplicated into both partition halves so two
            # k-chunks can run concurrently via PE row tiling.
            qt_sb = qk_pool.tile([128, s], F16, tag="qt")
            kt_sb = qk_pool.tile([128, s], F16, tag="kt")
            va_sb = v_pool.tile([128, n_chunks * (D + 1)], F16)
            va_r = va_d[h].rearrange("(c p) e -> p c e", p=128)
            # tiered loads: small leading slices of everything first (just
            # enough for the first block: 4 k-chunks + first qtile of Q),
            # split across both HWDGE queues (sync + scalar) so the critical
            # first-wave descriptors dispatch in parallel; then interleaved
            # k/V column pieces, with the q tails (needed only from qtile 1)
            # last
            kcut = min(4 * CHUNK, s)
            ncut = kcut // CHUNK
            nc.sync.dma_start(kt_sb[0:D, 0:kcut], kt_d[h][:, 0:kcut])
            nc.scalar.dma_start(kt_sb[D : 2 * D, 0:kcut], kt_d[h][:, 0:kcut])
            nc.sync.dma_start(qt_sb[0:D, 0:qtile], qt_d[h][:, 0:qtile])
            nc.scalar.dma_start(qt_sb[D : 2 * D, 0:qtile], qt_d[h][:, 0:qtile])
            nc.sync.dma_start(
                va_sb[:, 0 : ncut * (D + 1)], va_r[:, 0:ncut, :]
            )
            cuts = [kcut]
            while cuts[-1] < s:
                cuts.append(min(cuts[-1] + 12 * CHUNK, s))
            for c0_, c1_ in zip(cuts, cuts[1:]):
                n0_, n1_ = c0_ // CHUNK, c1_ // CHUNK
                nc.sync.dma_start(kt_sb[0:D, c0_:c1_], kt_d[h][:, c0_:c1_])
                nc.sync.dma_start(kt_sb[D : 2 * D, c0_:c1_], kt_d[h][:, c0_:c1_])
                nc.sync.dma_start(
                    va_sb[:, n0_ * (D + 1) : n1_ * (D + 1)], va_r[:, n0_:n1_, :]
                )
            if qtile < s:
                nc.sync.dma_start(qt_sb[0:D, qtile:s], qt_d[h][:, qtile:s])
                nc.sync.dma_start(qt_sb[D : 2 * D, qtile:s], qt_d[h][:, qtile:s])

            for qt in range(n_qtiles):
                qs = slice(qt * qtile, (qt + 1) * qtile)
                acc = oa_psum.tile([D + 1, qtile], F32)
                n_groups = len(group_sizes)
                # emit in blocks of two groups: MM1 x4, then both P ops (one
                # ACT + one DVE, concurrent), then older blocks' MM2s.
                # (3-group blocks were tried to amortize the ~97ns weight-
                # switch bubble over longer runs: slower overall -- with
                # sc bufs=3 a 3-group burst leaves zero pipeline slack and
                # the sc-recycle stalls cost more than the saved switches.)
                block_list = [
                    list(range(i, min(i + 2, n_groups)))
                    for i in range(0, n_groups, 2)
                ]
                for blk_groups in block_list:
                    blk = []
                    for gi in blk_groups:
                        gs = group_sizes[gi]
                        chunks = list(range(GROUP * gi, GROUP * gi + gs))
                        sc = sc_psum.tile([128, GROUP, qtile], F32, tag="sc")
                        for j, c in enumerate(chunks):
                            # alternate partition halves by global chunk index
                            # so consecutive chunks always run concurrently
                            # via PE row tiling, across group boundaries too
                            half = slice(0, D) if c % 2 == 0 else slice(D, 2 * D)
                            nc.tensor.matmul(
                                sc[:, j, :],
                                kt_sb[half, c * CHUNK : (c + 1) * CHUNK],
                                qt_sb[half, qs],
                                start=True, stop=True,
                            )
                        blk.append((gi, gs, chunks, sc))
                    for gi, gs, chunks, sc in blk:
                        p_t = pt_pool.tile([128, GROUP, qtile], F16, tag="pt")
                        if gi in DVE_GROUPS:
                            nc.vector.tensor_scalar(
                                p_t.bitcast(U16)[:, 0:gs, :], sc[:, 0:gs, :],
                                DVE_MUL, DVE_ADD, MULT, ADD,
                            )
                        else:
                            nc.scalar.activation(
                                p_t[:, 0:gs, :], sc[:, 0:gs, :], EXP, scale=0.125
                            )
                        pending.append((
                            h, qt, acc, chunks, p_t, va_sb,
                            gi == n_groups - 1,
                        ))
                    flush_pending(FLUSH_DEPTH)
        flush_pending()

    nc.compile()
    return nc


_NC_CACHE = {}


def _get_nc(hpc=HPC, s=S, qtile=QTILE):
    key = (hpc, s, qtile)
    if key not in _NC_CACHE:
        _NC_CACHE[key] = build_nc(hpc, s, qtile)
    return _NC_CACHE[key]


def prep_inputs(Q, K, V):
    """Host-side re-layout: per-core input maps."""
    bh = B * H
    q2 = np.ascontiguousarray(
        np.asarray(Q, dtype=np.float32).reshape(bh, S, D).transpose(0, 2, 1)
    ).astype(np.float16)
    k2 = np.ascontiguousarray(
        np.asarray(K, dtype=np.float32).reshape(bh, S, D).transpose(0, 2, 1)
    ).astype(np.float16)
    v = np.asarray(V, dtype=np.float32).reshape(bh, S, D).astype(np.float16)
    va = np.concatenate([v, np.ones((bh, S, 1), dtype=np.float16)], axis=-1)
    in_maps = []
    for c in range(N_CORES):
        sl = slice(c * HPC, (c + 1) * HPC)
        in_maps.append({
            "qt": np.ascontiguousarray(q2[sl]),
            "kt": np.ascontiguousarray(k2[sl]),
            "va": np.ascontiguousarray(va[sl]),
        })
    return in_maps


def run(Q, K, V, trace=False, **kwargs):
    nc = _get_nc()
    in_maps = prep_inputs(Q, K, V)
    res = run_bass_kernel_spmd(
        nc, in_maps, core_ids=list(range(N_CORES)), trace=trace, **kwargs
    )
    # o is [hpc, n_qtiles, D+1, qtile] fp32 raw accumulators; divide by the
    # Z row (softmax denominator) and transpose back to [hpc, s, D]
    outs = []
    for c in range(N_CORES):
        o = res.results[c]["o"]  # [hpc, nq, 65, qtile]
        out = o[:, :, 0:D, :] / o[:, :, D : D + 1, :]
        # [hpc, nq, D, qtile] -> [hpc, nq, qtile, D] -> [hpc, s, D]
        outs.append(out.transpose(0, 1, 3, 2).reshape(HPC, S, D))
    full = np.concatenate(outs, axis=0).reshape(B, H, S, D)
    return full, res


def kernel(Q, K, V):
    # retry on transient device/runtime errors (e.g. a wedged NeuronCore
    # left over from a previous run recovers on re-execution)
    import time
    last = None
    for attempt in range(3):
        try:
            out, _ = run(Q, K, V)
            return out
        except Exception as e:  # noqa: BLE001
            last = e
            time.sleep(5)
    raise last



# revision 29
# speedup vs baseline: 1.0094x; 1.0046x over previous
"""Multi-head attention kernel for Trainium2 (Bass/Tile), 8-core SPMD.

Problem: Q,K,V [B=2, H=16, S=4096, D=64] fp32 -> softmax(Q K^T / sqrt(D)) V.
Sharding: batch*heads (32) split 4-per-core across 8 NeuronCores; each core
computes its heads independently (no collectives).

Per-head algorithm (transposed-scores flash attention, fp16 matmuls, P
production split across two engines):
  scoresT[k,q] = K[k,:] . Q[q,:]        (PE, fp16 operands, fp32 PSUM,
                                         row-tiled pairs: two k-chunks run
                                         concurrently in the 128x128 array)
  pT[k,q]     = exp(scoresT / 8)        (fp16; per 3-chunk group either ACT
                                         exact exp, or DVE fp16-Schraudolph:
                                         u16 = round(s*1024*log2(e)/8 + b)
                                         bitcast as fp16 == 2^x with a
                                         mean-compensated ~+-3% mantissa-
                                         interpolation sawtooth that washes
                                         out in the softmax normalization)
  accT[d,q]  += Vaug[k,d] . pT[k,q]     (PE fp16; Vaug row 64 == ones, so acc
                                         row 64 accumulates the denominator)
  The raw accumulator [65, qtile] (numerator rows 0..63 + denominator row 64)
  is pulled PSUM->SBUF by one tracked DVE copy and DMA'd to DRAM in fp32;
  the softmax division happens on the HOST during the gather (removes the
  reciprocal/broadcast/multiply epilogue from the device critical path).
Host side re-lays-out data: QT/KT transposed per head, V augmented with
a ones column; output acc gathered, divided, transposed back.
"""

import numpy as np
from contextlib import ExitStack

import concourse.bacc as bacc
import concourse.bass as bass
import concourse.tile as tile
import concourse.mybir as mybir
from concourse.bass_utils import run_bass_kernel_spmd

F32 = mybir.dt.float32
F16 = mybir.dt.float16
U16 = mybir.dt.uint16
EXP = mybir.ActivationFunctionType.Exp
MULT = mybir.AluOpType.mult
ADD = mybir.AluOpType.add

B, H, S, D = 2, 16, 4096, 64
N_CORES = 8
HPC = (B * H) // N_CORES  # heads per core

QTILE = 512            # q columns processed per inner iteration
CHUNK = 128            # k rows per matmul (PE partition dim)
GROUP = 2              # k-chunks exp'd per ACT/DVE instruction

# fp16 Schraudolph: u16 = round(score * 1024*log2(e)/8 + bias); the bias is
# 15*1024 minus 58.7 to zero the mean of the 2^f vs (1+f) mantissa sawtooth
DVE_MUL = 184.66496
DVE_ADD = 15301.3
# groups (of 16 per qtile) whose P runs on DVE instead of ACT (16/32 chunks);
# ALL odd groups -> every 2-group block pairs exactly one ACT with one DVE
# producer (two consecutive same-engine groups serialize ~2.2us of exp in a
# ~1.5us block and stall the next qtile's MM1 on the sc-buffer recycle)
DVE_GROUPS = frozenset({1, 3, 5, 7, 9, 11, 13, 15})
FLUSH_DEPTH = 2        # MM2 groups held back so PE never queues behind P


def build_nc(hpc: int = HPC, s: int = S, qtile: int = QTILE):
    n_chunks = s // CHUNK
    n_qtiles = s // qtile
    group_sizes = [GROUP] * (n_chunks // GROUP)
    if n_chunks % GROUP:
        group_sizes.append(n_chunks % GROUP)

    nc = bacc.Bacc("TRN2", target_bir_lowering=False, debug=False)
    qt_d = nc.dram_tensor("qt", [hpc, D, s], F16, kind="ExternalInput").ap()
    kt_d = nc.dram_tensor("kt", [hpc, D, s], F16, kind="ExternalInput").ap()
    va_d = nc.dram_tensor("va", [hpc, s, D + 1], F16, kind="ExternalInput").ap()
    o_d = nc.dram_tensor(
        "o", [hpc, s // qtile, D + 1, qtile], F32, kind="ExternalOutput"
    ).ap()

    with tile.TileContext(nc) as tc, ExitStack() as ctx:
        qk_pool = ctx.enter_context(tc.tile_pool(name="qk", bufs=2))
        v_pool = ctx.enter_context(tc.tile_pool(name="v", bufs=2))
        pt_pool = ctx.enter_context(tc.tile_pool(name="pt", bufs=6))
        oc_pool = ctx.enter_context(tc.tile_pool(name="oc", bufs=3))
        const_pool = ctx.enter_context(tc.tile_pool(name="const", bufs=1))
        sc_psum = ctx.enter_context(tc.tile_pool(name="sc", bufs=3, space="PSUM"))
        oa_psum = ctx.enter_context(tc.tile_pool(name="oa", bufs=2, space="PSUM"))

        # prewarm the PE HAM clock gate: the hardware activity monitor only
        # releases the 1.2->2.4 GHz throttle after a full ~3.4us window of
        # sustained PE activity, which otherwise burns the first ~dozens of
        # real matmuls at half clock; ~4.5us of junk micro-matmuls into a
        # scratch PSUM tile keep the PE busy through the initial DMA window
        # so real MM1s start warm
        wj = const_pool.tile([128, 128], F16)
        nc.vector.memset(wj[:], 1.0)
        # prewarm the ACT exp table set while the first DMAs are in flight
        warm = const_pool.tile([1, 1], F32)
        nc.vector.memset(warm[:], 0.0)
        warm2 = const_pool.tile([1, 1], F32)
        nc.scalar.activation(warm2[:], warm[:], EXP, scale=1.0)
        # full-array (128x128) junk matmuls: the HAM watches array
        # utilization, so narrow warmup MMs don't register as busy
        # (tag "sc": shares the sc ring's buffers, so no extra PSUM banks)
        warm_ps = sc_psum.tile([128, 128], F32, tag="sc")
        for _ in range(42):
            nc.tensor.matmul(
                warm_ps[:], wj[:], wj[:], start=True, stop=True,
                skip_group_check=True,
            )

        # software-pipelined emission: each group's PV matmuls (MM2) are
        # deferred until after the NEXT group's score matmuls (MM1) and P
        # production -- across qtile AND head boundaries -- so the next P is
        # never queued on PE behind MM2s that wait on the current P.
        pending = []  # deque of (h, qt, acc, chunks, p_t, va_sb, is_last)

        def epilogue(h_, qt_, acc_):
            # pull the raw accumulator (numerator + Z row) out of PSUM with a
            # *tracked* DVE copy (waits for the accumulation stop), then ship
            # it fp32 to DRAM on the HWDGE ring; the softmax division happens
            # host-side during the gather.
            oc = oc_pool.tile([D + 1, qtile], F32, tag="oc")
            if h_ == hpc - 1 and qt_ == s // qtile - 1:
                # final qtile is the serial tail of the whole kernel: split
                # the copy/DMA in half so the first DMA overlaps the second
                # copy
                half = qtile // 2
                nc.vector.tensor_copy(oc[:, 0:half], acc_[:, 0:half])
                nc.sync.dma_start(o_d[h_][qt_][:, 0:half], oc[:, 0:half])
                nc.vector.tensor_copy(oc[:, half:], acc_[:, half:])
                nc.sync.dma_start(o_d[h_][qt_][:, half:], oc[:, half:])
            else:
                nc.vector.tensor_copy(oc[:], acc_[:])
                nc.sync.dma_start(o_d[h_][qt_], oc[:])

        def flush_one():
            # NOTE: padding the weight AP to a full 128 columns (output
            # partitions 65..127 as junk) was tried to get fast-weight-load:
            # it makes the whole PE run 1.2x SLOWER -- the extra array
            # columns burn real power and trip the P0 downclock (2.4->2.0
            # GHz). Keep the weight tile at its true 65 columns.
            h_, qt_, acc_, chunks_, pt_, va_, last_ = pending.pop(0)
            for j, c in enumerate(chunks_):
                nc.tensor.matmul(
                    acc_[:],
                    va_[:, c * (D + 1) : (c + 1) * (D + 1)],
                    pt_[:, j, :],
                    start=(c == 0), stop=(c == n_chunks - 1),
                )
            if last_:
                epilogue(h_, qt_, acc_)

        def flush_pending(depth=0):
            while len(pending) > depth:
                flush_one()

        for h in range(hpc):
            # K^T and Q^T [D, s] duplicated into both partition halves so two
            # k-chunks can run concurrently via PE row tiling.
            qt_sb = qk_pool.tile([128, s], F16, tag="qt")
            kt_sb = qk_pool.tile([128, s], F16, tag="kt")
            va_sb = v_pool.tile([128, n_chunks * (D + 1)], F16)
            va_r = va_d[h].rearrange("(c p) e -> p c e", p=128)
            # tiered loads: small leading slices of everything first (just
            # enough for the first block: 4 k-chunks + first qtile of Q),
            # split across both HWDGE queues (sync + scalar) so the critical
            # first-wave descriptors dispatch in parallel; then interleaved
            # k/V column pieces, with the q tails (needed only from qtile 1)
            # last
            kcut = min(4 * CHUNK, s)
            ncut = kcut // CHUNK
            nc.sync.dma_start(kt_sb[0:D, 0:kcut], kt_d[h][:, 0:kcut])
            nc.scalar.dma_start(kt_sb[D : 2 * D, 0:kcut], kt_d[h][:, 0:kcut])
            nc.sync.dma_start(qt_sb[0:D, 0:qtile], qt_d[h][:, 0:qtile])
            nc.scalar.dma_start(qt_sb[D : 2 * D, 0:qtile], qt_d[h][:, 0:qtile])
            nc.sync.dma_start(
                va_sb[:, 0 : ncut * (D + 1)], va_r[:, 0:ncut, :]
            )
            # doubling tiers (4,4,8,16 chunks): the chunk loop consumes k
            # columns in order, and a block stalls until the WHOLE tier DMA
            # containing its chunks lands -- small early tiers keep the
            # first blocks' waits short (a 12-chunk tier stalled block 2 by
            # ~1.5us and let the HAM clock gate re-throttle the PE)
            cuts = [kcut]
            while cuts[-1] < s:
                cuts.append(min(2 * cuts[-1], s))
            for c0_, c1_ in zip(cuts, cuts[1:]):
                n0_, n1_ = c0_ // CHUNK, c1_ // CHUNK
                nc.sync.dma_start(kt_sb[0:D, c0_:c1_], kt_d[h][:, c0_:c1_])
                nc.sync.dma_start(kt_sb[D : 2 * D, c0_:c1_], kt_d[h][:, c0_:c1_])
                nc.sync.dma_start(
                    va_sb[:, n0_ * (D + 1) : n1_ * (D + 1)], va_r[:, n0_:n1_, :]
                )
            if qtile < s:
                nc.sync.dma_start(qt_sb[0:D, qtile:s], qt_d[h][:, qtile:s])
                nc.sync.dma_start(qt_sb[D : 2 * D, qtile:s], qt_d[h][:, qtile:s])

            for qt in range(n_qtiles):
                qs = slice(qt * qtile, (qt + 1) * qtile)
                acc = oa_psum.tile([D + 1, qtile], F32)
                n_groups = len(group_sizes)
                # emit in blocks of two groups: MM1 x4, then both P ops (one
                # ACT + one DVE, concurrent), then older blocks' MM2s.
                # (3-group blocks were tried to amortize the ~97ns weight-
                # switch bubble over longer runs: slower overall -- with
                # sc bufs=3 a 3-group burst leaves zero pipeline slack and
                # the sc-recycle stalls cost more than the saved switches.)
                block_list = [
                    list(range(i, min(i + 2, n_groups)))
                    for i in range(0, n_groups, 2)
                ]
                for blk_groups in block_list:
                    blk = []
                    for gi in blk_groups:
                        gs = group_sizes[gi]
                        chunks = list(range(GROUP * gi, GROUP * gi + gs))
                        sc = sc_psum.tile([128, GROUP, qtile], F32, tag="sc")
                        for j, c in enumerate(chunks):
                            # alternate partition halves by global chunk index
                            # so consecutive chunks always run concurrently
                            # via PE row tiling, across group boundaries too
                            half = slice(0, D) if c % 2 == 0 else slice(D, 2 * D)
                            nc.tensor.matmul(
                                sc[:, j, :],
                                kt_sb[half, c * CHUNK : (c + 1) * CHUNK],
                                qt_sb[half, qs],
                                start=True, stop=True,
                            )
                        blk.append((gi, gs, chunks, sc))
                    for gi, gs, chunks, sc in blk:
                        p_t = pt_pool.tile([128, GROUP, qtile], F16, tag="pt")
                        if gi in DVE_GROUPS:
                            nc.vector.tensor_scalar(
                                p_t.bitcast(U16)[:, 0:gs, :], sc[:, 0:gs, :],
                                DVE_MUL, DVE_ADD, MULT, ADD,
                            )
                        else:
                            nc.scalar.activation(
                                p_t[:, 0:gs, :], sc[:, 0:gs, :], EXP, scale=0.125
                            )
                        pending.append((
                            h, qt, acc, chunks, p_t, va_sb,
                            gi == n_groups - 1,
                        ))
                    flush_pending(FLUSH_DEPTH)
        flush_pending()

    nc.compile()
    return nc


_NC_CACHE = {}


def _get_nc(hpc=HPC, s=S, qtile=QTILE):
    key = (hpc, s, qtile)
    if key not in _NC_CACHE:
        _NC_CACHE[key] = build_nc(hpc, s, qtile)
    return _NC_CACHE[key]


def prep_inputs(Q, K, V):
    """Host-side re-layout: per-core input maps."""
    bh = B * H
    q2 = np.ascontiguousarray(
        np.asarray(Q, dtype=np.float32).reshape(bh, S, D).transpose(0, 2, 1)
    ).astype(np.float16)
    k2 = np.ascontiguousarray(
        np.asarray(K, dtype=np.float32).reshape(bh, S, D).transpose(0, 2, 1)
    ).astype(np.float16)
    v = np.asarray(V, dtype=np.float32).reshape(bh, S, D).astype(np.float16)
    va = np.concatenate([v, np.ones((bh, S, 1), dtype=np.float16)], axis=-1)
    in_maps = []
    for c in range(N_CORES):
        sl = slice(c * HPC, (c + 1) * HPC)
        in_maps.append({
            "qt": np.ascontiguousarray(q2[sl]),
            "kt": np.ascontiguousarray(k2[sl]),
            "va": np.ascontiguousarray(va[sl]),
        })
    return in_maps


def run(Q, K, V, trace=False, **kwargs):
    nc = _get_nc()
    in_maps = prep_inputs(Q, K, V)
    res = run_bass_kernel_spmd(
        nc, in_maps, core_ids=list(range(N_CORES)), trace=trace, **kwargs
    )
    # o is [hpc, n_qtiles, D+1, qtile] fp32 raw accumulators; divide by the
    # Z row (softmax denominator) and transpose back to [hpc, s, D]
    outs = []
    for c in range(N_CORES):
        o = res.results[c]["o"]  # [hpc, nq, 65, qtile]
        out = o[:, :, 0:D, :] / o[:, :, D : D + 1, :]
        # [hpc, nq, D, qtile] -> [hpc, nq, qtile, D] -> [hpc, s, D]
        outs.append(out.transpose(0, 1, 3, 2).reshape(HPC, S, D))
    full = np.concatenate(outs, axis=0).reshape(B, H, S, D)
    return full, res


def kernel(Q, K, V):
    # retry on transient device/runtime errors (e.g. a wedged NeuronCore
    # left over from a previous run recovers on re-execution)
    import time
    last = None
    for attempt in range(3):
        try:
            out, _ = run(Q, K, V)
            return out
        except Exception as e:  # noqa: BLE001
            last = e
            time.sleep(5)
    raise last



# revision 30
# speedup vs baseline: 1.0109x; 1.0015x over previous
"""Multi-head attention kernel for Trainium2 (Bass/Tile), 8-core SPMD.

Problem: Q,K,V [B=2, H=16, S=4096, D=64] fp32 -> softmax(Q K^T / sqrt(D)) V.
Sharding: batch*heads (32) split 4-per-core across 8 NeuronCores; each core
computes its heads independently (no collectives).

Per-head algorithm (transposed-scores flash attention, fp16 matmuls, P
production split across two engines):
  scoresT[k,q] = K[k,:] . Q[q,:]        (PE, fp16 operands, fp32 PSUM,
                                         row-tiled pairs: two k-chunks run
                                         concurrently in the 128x128 array)
  pT[k,q]     = exp(scoresT / 8)        (fp16; per 3-chunk group either ACT
                                         exact exp, or DVE fp16-Schraudolph:
                                         u16 = round(s*1024*log2(e)/8 + b)
                                         bitcast as fp16 == 2^x with a
                                         mean-compensated ~+-3% mantissa-
                                         interpolation sawtooth that washes
                                         out in the softmax normalization)
  accT[d,q]  += Vaug[k,d] . pT[k,q]     (PE fp16; Vaug row 64 == ones, so acc
                                         row 64 accumulates the denominator)
  The raw accumulator [65, qtile] (numerator rows 0..63 + denominator row 64)
  is pulled PSUM->SBUF by one tracked DVE copy and DMA'd to DRAM in fp32;
  the softmax division happens on the HOST during the gather (removes the
  reciprocal/broadcast/multiply epilogue from the device critical path).
Host side re-lays-out data: QT/KT transposed per head, V augmented with
a ones column; output acc gathered, divided, transposed back.
"""

import numpy as np
from contextlib import ExitStack

import concourse.bacc as bacc
import concourse.bass as bass
import concourse.tile as tile
import concourse.mybir as mybir
from concourse.bass_utils import run_bass_kernel_spmd

F32 = mybir.dt.float32
F16 = mybir.dt.float16
U16 = mybir.dt.uint16
EXP = mybir.ActivationFunctionType.Exp
MULT = mybir.AluOpType.mult
ADD = mybir.AluOpType.add

B, H, S, D = 2, 16, 4096, 64
N_CORES = 8
HPC = (B * H) // N_CORES  # heads per core

QTILE = 512            # q columns processed per inner iteration
CHUNK = 128            # k rows per matmul (PE partition dim)
GROUP = 2              # k-chunks exp'd per ACT/DVE instruction

# fp16 Schraudolph: u16 = round(score * 1024*log2(e)/8 + bias); the bias is
# 15*1024 minus 58.7 to zero the mean of the 2^f vs (1+f) mantissa sawtooth
DVE_MUL = 184.66496
DVE_ADD = 15301.3
# groups (of 16 per qtile) whose P runs on DVE instead of ACT (16/32 chunks);
# ALL odd groups -> every 2-group block pairs exactly one ACT with one DVE
# producer (two consecutive same-engine groups serialize ~2.2us of exp in a
# ~1.5us block and stall the next qtile's MM1 on the sc-buffer recycle)
DVE_GROUPS = frozenset({1, 3, 5, 7, 9, 11, 13, 15})
FLUSH_DEPTH = 2        # MM2 groups held back so PE never queues behind P


def build_nc(hpc: int = HPC, s: int = S, qtile: int = QTILE):
    n_chunks = s // CHUNK
    n_qtiles = s // qtile
    group_sizes = [GROUP] * (n_chunks // GROUP)
    if n_chunks % GROUP:
        group_sizes.append(n_chunks % GROUP)

    nc = bacc.Bacc("TRN2", target_bir_lowering=False, debug=False)
    qt_d = nc.dram_tensor("qt", [hpc, D, s], F16, kind="ExternalInput").ap()
    kt_d = nc.dram_tensor("kt", [hpc, D, s], F16, kind="ExternalInput").ap()
    va_d = nc.dram_tensor("va", [hpc, s, D + 1], F16, kind="ExternalInput").ap()
    o_d = nc.dram_tensor(
        "o", [hpc, s // qtile, D + 1, qtile], F32, kind="ExternalOutput"
    ).ap()

    with tile.TileContext(nc) as tc, ExitStack() as ctx:
        qk_pool = ctx.enter_context(tc.tile_pool(name="qk", bufs=2))
        v_pool = ctx.enter_context(tc.tile_pool(name="v", bufs=2))
        pt_pool = ctx.enter_context(tc.tile_pool(name="pt", bufs=6))
        oc_pool = ctx.enter_context(tc.tile_pool(name="oc", bufs=3))
        const_pool = ctx.enter_context(tc.tile_pool(name="const", bufs=1))
        sc_psum = ctx.enter_context(tc.tile_pool(name="sc", bufs=3, space="PSUM"))
        oa_psum = ctx.enter_context(tc.tile_pool(name="oa", bufs=2, space="PSUM"))

        # prewarm the PE HAM clock gate: the hardware activity monitor only
        # releases the 1.2->2.4 GHz throttle after a full ~3.4us window of
        # sustained PE activity, which otherwise burns the first ~dozens of
        # real matmuls at half clock; ~4.5us of junk micro-matmuls into a
        # scratch PSUM tile keep the PE busy through the initial DMA window
        # so real MM1s start warm
        wj = const_pool.tile([128, 128], F16)
        nc.vector.memset(wj[:], 1.0)
        # prewarm the ACT exp table set while the first DMAs are in flight
        warm = const_pool.tile([1, 1], F32)
        nc.vector.memset(warm[:], 0.0)
        warm2 = const_pool.tile([1, 1], F32)
        nc.scalar.activation(warm2[:], warm[:], EXP, scale=1.0)
        # full-array (128x128) junk matmuls: the HAM watches array
        # utilization, so narrow warmup MMs don't register as busy
        # (tag "sc": shares the sc ring's buffers, so no extra PSUM banks)
        warm_ps = sc_psum.tile([128, 128], F32, tag="sc")
        for _ in range(42):
            nc.tensor.matmul(
                warm_ps[:], wj[:], wj[:], start=True, stop=True,
                skip_group_check=True,
            )

        # software-pipelined emission: each group's PV matmuls (MM2) are
        # deferred until after the NEXT group's score matmuls (MM1) and P
        # production -- across qtile AND head boundaries -- so the next P is
        # never queued on PE behind MM2s that wait on the current P.
        pending = []  # deque of (h, qt, acc, chunks, p_t, va_sb, is_last)

        def epilogue(h_, qt_, acc_):
            # pull the raw accumulator (numerator + Z row) out of PSUM with a
            # *tracked* DVE copy (waits for the accumulation stop), then ship
            # it fp32 to DRAM on the HWDGE ring; the softmax division happens
            # host-side during the gather.
            oc = oc_pool.tile([D + 1, qtile], F32, tag="oc")
            if h_ == hpc - 1 and qt_ == s // qtile - 1:
                # final qtile is the serial tail of the whole kernel: split
                # the copy/DMA in half so the first DMA overlaps the second
                # copy
                half = qtile // 2
                nc.vector.tensor_copy(oc[:, 0:half], acc_[:, 0:half])
                nc.sync.dma_start(o_d[h_][qt_][:, 0:half], oc[:, 0:half])
                nc.vector.tensor_copy(oc[:, half:], acc_[:, half:])
                nc.sync.dma_start(o_d[h_][qt_][:, half:], oc[:, half:])
            else:
                nc.vector.tensor_copy(oc[:], acc_[:])
                nc.sync.dma_start(o_d[h_][qt_], oc[:])

        def flush_one():
            # NOTE: padding the weight AP to a full 128 columns (output
            # partitions 65..127 as junk) was tried to get fast-weight-load:
            # it makes the whole PE run 1.2x SLOWER -- the extra array
            # columns burn real power and trip the P0 downclock (2.4->2.0
            # GHz). Keep the weight tile at its true 65 columns.
            h_, qt_, acc_, chunks_, pt_, va_, last_ = pending.pop(0)
            for j, c in enumerate(chunks_):
                nc.tensor.matmul(
                    acc_[:],
                    va_[:, c * (D + 1) : (c + 1) * (D + 1)],
                    pt_[:, j, :],
                    start=(c == 0), stop=(c == n_chunks - 1),
                )
            if last_:
                epilogue(h_, qt_, acc_)

        def flush_pending(depth=0):
            while len(pending) > depth:
                flush_one()

        for h in range(hpc):
            # K^T and Q^T [D, s] duplicated into both partition halves so two
            # k-chunks can run concurrently via PE row tiling.
            qt_sb = qk_pool.tile([128, s], F16, tag="qt")
            kt_sb = qk_pool.tile([128, s], F16, tag="kt")
            va_sb = v_pool.tile([128, n_chunks * (D + 1)], F16)
            va_r = va_d[h].rearrange("(c p) e -> p c e", p=128)
            # tiered loads: small leading slices of everything first (just
            # enough for the first block: 4 k-chunks + first qtile of Q),
            # split across both HWDGE queues (sync + scalar) so the critical
            # first-wave descriptors dispatch in parallel; then interleaved
            # k/V column pieces, with the q tails (needed only from qtile 1)
            # last
            kcut = min(4 * CHUNK, s)
            ncut = kcut // CHUNK
            nc.sync.dma_start(kt_sb[0:D, 0:kcut], kt_d[h][:, 0:kcut])
            nc.scalar.dma_start(kt_sb[D : 2 * D, 0:kcut], kt_d[h][:, 0:kcut])
            nc.sync.dma_start(qt_sb[0:D, 0:qtile], qt_d[h][:, 0:qtile])
            nc.scalar.dma_start(qt_sb[D : 2 * D, 0:qtile], qt_d[h][:, 0:qtile])
            nc.sync.dma_start(
                va_sb[:, 0 : ncut * (D + 1)], va_r[:, 0:ncut, :]
            )
            # doubling tiers (4,4,8,16 chunks): the chunk loop consumes k
            # columns in order, and a block stalls until the WHOLE tier DMA
            # containing its chunks lands -- small early tiers keep the
            # first blocks' waits short (a 12-chunk tier stalled block 2 by
            # ~1.5us and let the HAM clock gate re-throttle the PE)
            cuts = [kcut]
            while cuts[-1] < s:
                cuts.append(min(2 * cuts[-1], s))
            # head 0 races the PE against cold DMA: split the two kt halves
            # across both HWDGE queues there (scalar queue is safe only on
            # head 0, where these loads wait on nothing and cannot
            # head-block the ACT exp FIFO)
            kt_eng2 = nc.scalar if h == 0 else nc.sync
            for c0_, c1_ in zip(cuts, cuts[1:]):
                n0_, n1_ = c0_ // CHUNK, c1_ // CHUNK
                nc.sync.dma_start(kt_sb[0:D, c0_:c1_], kt_d[h][:, c0_:c1_])
                kt_eng2.dma_start(kt_sb[D : 2 * D, c0_:c1_], kt_d[h][:, c0_:c1_])
                nc.sync.dma_start(
                    va_sb[:, n0_ * (D + 1) : n1_ * (D + 1)], va_r[:, n0_:n1_, :]
                )
            if qtile < s:
                nc.sync.dma_start(qt_sb[0:D, qtile:s], qt_d[h][:, qtile:s])
                nc.sync.dma_start(qt_sb[D : 2 * D, qtile:s], qt_d[h][:, qtile:s])

            for qt in range(n_qtiles):
                qs = slice(qt * qtile, (qt + 1) * qtile)
                acc = oa_psum.tile([D + 1, qtile], F32)
                n_groups = len(group_sizes)
                # emit in blocks of two groups: MM1 x4, then both P ops (one
                # ACT + one DVE, concurrent), then older blocks' MM2s.
                # (3-group blocks were tried to amortize the ~97ns weight-
                # switch bubble over longer runs: slower overall -- with
                # sc bufs=3 a 3-group burst leaves zero pipeline slack and
                # the sc-recycle stalls cost more than the saved switches.)
                block_list = [
                    list(range(i, min(i + 2, n_groups)))
                    for i in range(0, n_groups, 2)
                ]
                for blk_groups in block_list:
                    blk = []
                    for gi in blk_groups:
                        gs = group_sizes[gi]
                        chunks = list(range(GROUP * gi, GROUP * gi + gs))
                        sc = sc_psum.tile([128, GROUP, qtile], F32, tag="sc")
                        for j, c in enumerate(chunks):
                            # alternate partition halves by global chunk index
                            # so consecutive chunks always run concurrently
                            # via PE row tiling, across group boundaries too
                            half = slice(0, D) if c % 2 == 0 else slice(D, 2 * D)
                            nc.tensor.matmul(
                                sc[:, j, :],
                                kt_sb[half, c * CHUNK : (c + 1) * CHUNK],
                                qt_sb[half, qs],
                                start=True, stop=True,
                            )
                        blk.append((gi, gs, chunks, sc))
                    for gi, gs, chunks, sc in blk:
                        p_t = pt_pool.tile([128, GROUP, qtile], F16, tag="pt")
                        if gi in DVE_GROUPS:
                            nc.vector.tensor_scalar(
                                p_t.bitcast(U16)[:, 0:gs, :], sc[:, 0:gs, :],
                                DVE_MUL, DVE_ADD, MULT, ADD,
                            )
                        else:
                            nc.scalar.activation(
                                p_t[:, 0:gs, :], sc[:, 0:gs, :], EXP, scale=0.125
                            )
                        pending.append((
                            h, qt, acc, chunks, p_t, va_sb,
                            gi == n_groups - 1,
                        ))
                    flush_pending(FLUSH_DEPTH)
        flush_pending()

    nc.compile()
    return nc


_NC_CACHE = {}


def _get_nc(hpc=HPC, s=S, qtile=QTILE):
    key = (hpc, s, qtile)
    if key not in _NC_CACHE:
        _NC_CACHE[key] = build_nc(hpc, s, qtile)
    return _NC_CACHE[key]


def prep_inputs(Q, K, V):
    """Host-side re-layout: per-core input maps."""
    bh = B * H
    q2 = np.ascontiguousarray(
        np.asarray(Q, dtype=np.float32).reshape(bh, S, D).transpose(0, 2, 1)
    ).astype(np.float16)
    k2 = np.ascontiguousarray(
        np.asarray(K, dtype=np.float32).reshape(bh, S, D).transpose(0, 2, 1)
    ).astype(np.float16)
    v = np.asarray(V, dtype=np.float32).reshape(bh, S, D).astype(np.float16)
    va = np.concatenate([v, np.ones((bh, S, 1), dtype=np.float16)], axis=-1)
    in_maps = []
    for c in range(N_CORES):
        sl = slice(c * HPC, (c + 1) * HPC)
        in_maps.append({
            "qt": np.ascontiguousarray(q2[sl]),
            "kt": np.ascontiguousarray(k2[sl]),
            "va": np.ascontiguousarray(va[sl]),
        })
    return in_maps


def run(Q, K, V, trace=False, **kwargs):
    nc = _get_nc()
    in_maps = prep_inputs(Q, K, V)
    res = run_bass_kernel_spmd(
        nc, in_maps, core_ids=list(range(N_CORES)), trace=trace, **kwargs
    )
    # o is [hpc, n_qtiles, D+1, qtile] fp32 raw accumulators; divide by the
    # Z row (softmax denominator) and transpose back to [hpc, s, D]
    outs = []
    for c in range(N_CORES):
        o = res.results[c]["o"]  # [hpc, nq, 65, qtile]
        out = o[:, :, 0:D, :] / o[:, :, D : D + 1, :]
        # [hpc, nq, D, qtile] -> [hpc, nq, qtile, D] -> [hpc, s, D]
        outs.append(out.transpose(0, 1, 3, 2).reshape(HPC, S, D))
    full = np.concatenate(outs, axis=0).reshape(B, H, S, D)
    return full, res


def kernel(Q, K, V):
    # retry on transient device/runtime errors (e.g. a wedged NeuronCore
    # left over from a previous run recovers on re-execution)
    import time
    last = None
    for attempt in range(3):
        try:
            out, _ = run(Q, K, V)
            return out
        except Exception as e:  # noqa: BLE001
            last = e
            time.sleep(5)
    raise last

